# revision 4
# baseline (speedup 1.0000x reference)
import numpy as np
from concourse import bacc, tile, mybir
from concourse import bass_utils

B, N, D, K = 8, 2048, 256, 16
EPS = 1e-5
CH = 128           # tokens per row-tile
RT = N // CH       # 16 row-tiles
FC = 512           # pairs per PSUM block
NFC = CH * K // FC # 4 blocks per row-tile
TPF = FC // K      # tokens per fc block (32)

f32 = mybir.dt.float32
f32r = mybir.dt.float32r
bf16 = mybir.dt.bfloat16
u16 = mybir.dt.uint16
i16 = mybir.dt.int16
AF = mybir.ActivationFunctionType
AX = mybir.AxisListType
OP = mybir.AluOpType

REDUCE_GPSIMD = True   # K-reductions on gpsimd (else DVE)
RECIP_FAST = True

_NC = None


def _build():
    nc = bacc.Bacc("TRN2", target_bir_lowering=False, debug=False)
    xT_d = nc.dram_tensor("xT", [D, N], f32, kind="ExternalInput")
    pp_d = nc.dram_tensor("pospack", [8, N], f32, kind="ExternalInput")
    wproj_d = nc.dram_tensor("wproj", [D, 768], f32, kind="ExternalInput")
    wpair_d = nc.dram_tensor("wpair", [D, 768], f32, kind="ExternalInput")
    wf_d = nc.dram_tensor("wfin", [D, 256], f32, kind="ExternalInput")
    sw_d = nc.dram_tensor("spw1T", [4, D], f32, kind="ExternalInput")
    bs_d = nc.dram_tensor("biases", [128, 6], f32, kind="ExternalInput")
    rep16_d = nc.dram_tensor("rep16", [128, 128], f32, kind="ExternalInput")
    rep4n_d = nc.dram_tensor("rep4n", [4, 128], f32, kind="ExternalInput")
    ident_d = nc.dram_tensor("ident", [128, 128], f32, kind="ExternalInput")
    outT_d = nc.dram_tensor("outT", [D, N], f32, kind="ExternalOutput")

    with tile.TileContext(nc) as tc:
        with tc.tile_pool(name="pers", bufs=1) as P, tc.tile_pool(
            name="rtp", bufs=1
        ) as R, tc.tile_pool(name="gp", bufs=2) as GP, tc.tile_pool(
            name="fp", bufs=2
        ) as FP, tc.tile_pool(name="ps", bufs=1, space="PSUM") as PS:
            # ---------------- persistent loads ----------------
            xs = [P.tile([128, N], f32r, name=f"xs{h}") for h in range(2)]
            wproj = [P.tile([128, 768], f32r, name=f"wproj{h}") for h in range(2)]
            for h in range(2):
                nc.sync.dma_start(xs[h][:], xT_d[128 * h : 128 * (h + 1), :].bitcast(f32r))
                nc.sync.dma_start(wproj[h][:], wproj_d[128 * h : 128 * (h + 1), :].bitcast(f32r))
            ppA = P.tile([4, N], f32, name="ppA")
            nc.sync.dma_start(ppA[:], pp_d[0:4, :])
            ppB = P.tile([4, N], f32, name="ppB")
            nc.sync.dma_start(ppB[:], pp_d[4:8, :])
            bs = P.tile([128, 6], f32, name="bs")
            nc.sync.dma_start(bs[:], bs_d[:])
            rep16 = P.tile([128, 128], f32, name="rep16")
            nc.sync.dma_start(rep16[:], rep16_d[:])
            rep4n = P.tile([4, 128], f32, name="rep4n")
            nc.sync.dma_start(rep4n[:], rep4n_d[:])
            ident = P.tile([128, 128], f32, name="ident")
            nc.sync.dma_start(ident[:], ident_d[:])

            # bf16 weights (device-side conversion from f32)
            wpair = [P.tile([128, 768], bf16, name=f"wpair{h}") for h in range(2)]
            wf = [P.tile([128, 256], bf16, name=f"wf{h}") for h in range(2)]
            sw = P.tile([4, D], bf16, name="sw")
            identb = P.tile([128, 128], bf16, name="identb")
            ppAb = P.tile([4, N], bf16, name="ppAb")
            with tc.tile_pool(name="cvt", bufs=1) as CV:
                for h in range(2):
                    t = CV.tile([128, 768], f32, name=f"cw{h}", tag="cw")
                    nc.sync.dma_start(t[:], wpair_d[128 * h : 128 * (h + 1), :])
                    nc.scalar.copy(wpair[h][:], t[:])
                    t2 = CV.tile([128, 256], f32, name=f"cf{h}", tag="cf")
                    nc.sync.dma_start(t2[:], wf_d[128 * h : 128 * (h + 1), :])
                    nc.scalar.copy(wf[h][:], t2[:])
                t3 = CV.tile([4, D], f32, name="csw")
                nc.sync.dma_start(t3[:], sw_d[:])
                nc.scalar.copy(sw[:], t3[:])
                nc.scalar.copy(identb[:], ident[:])
                nc.scalar.copy(ppAb[:], ppA[:])

            # ---------------- projections + packed table ----------------
            # kvu6[p, m, slot]: (-k~0, -k~1, v0, v1, -aux, pad)  bf16
            # (k~ negation via negated Wk in wproj; aux negation via rep4n)
            kvu6 = P.tile([128, N * 6], bf16, name="kvu6")
            kvu6v = kvu6[:].rearrange("p (n d) -> p n d", d=6)
            qb = [P.tile([128, N], bf16, name=f"qb{h}") for h in range(2)]

            # aux slot: -pos component (p%4) replicated to 128 partitions
            for nf in range(4):
                pb = PS.tile([128, 512], f32, name="pb", tag="psA", bufs=2)
                nc.tensor.matmul(
                    pb[:], rep4n[:], ppA[:, nf * 512 : (nf + 1) * 512],
                    start=True, stop=True,
                )
                nc.scalar.copy(kvu6v[:, nf * 512 : (nf + 1) * 512, 4], pb[:])

            # q~ = (sam_w1@Wq)@x + biasA ; -k~ = -(sam_w1@Wk)@x ; v = Wv@x
            for o in range(3):
                for h in range(2):
                    col = o * 256 + h * 128
                    for nf in range(4):
                        pA = PS.tile([128, 512], f32, name="pA", tag="psA", bufs=2)
                        nc.tensor.matmul(
                            pA[:], wproj[0][:, col : col + 128],
                            xs[0][:, nf * 512 : (nf + 1) * 512],
                            start=True, stop=False,
                        )
                        nc.tensor.matmul(
                            pA[:], wproj[1][:, col : col + 128],
                            xs[1][:, nf * 512 : (nf + 1) * 512],
                            start=False, stop=True,
                        )
                        sl = slice(nf * 512, (nf + 1) * 512)
                        if o == 0:
                            nc.scalar.activation(
                                qb[h][:, sl], pA[:], AF.Identity,
                                bias=bs[:, 4 + h : 5 + h], scale=1.0,
                            )
                        elif o == 1:
                            nc.scalar.copy(kvu6v[:, sl, h], pA[:])
                        else:
                            nc.scalar.copy(kvu6v[:, sl, 2 + h], pA[:])

            # ---------------- per-row-tile state ----------------
            idxts = [R.tile([128, 128], i16, name=f"idxt{i}", tag=f"idxt{i}")
                     for i in range(2)]

            def scores_topk(rt):
                t0 = rt * CH
                Gs = R.tile([128, 2048], f32, name="Gs", tag="Gs")
                for bb in range(4):
                    Gp = PS.tile([128, 512], f32, name="Gp", tag="psA", bufs=2)
                    nc.tensor.matmul(
                        Gp[:], ppB[:, t0 : t0 + 128],
                        ppA[:, bb * 512 : (bb + 1) * 512],
                        start=True, stop=True,
                    )
                    nc.scalar.copy(Gs[:, bb * 512 : (bb + 1) * 512], Gp[:])
                mxt = R.tile([128, 16], f32, name="mxt", tag="mxt")
                mip = R.tile([128, 128], u16, name="mip", tag="mip")
                nc.gpsimd.memset(mip[:], 0)
                nc.vector.max(mxt[:, 0:8], Gs[:])
                nc.vector.max_index(mip[:, 0:8], mxt[:, 0:8], Gs[:])
                Gm = R.tile([128, 2048], f32, name="Gm", tag="Gm")
                nc.vector.match_replace(Gm[:], mxt[:, 0:8], Gs[:], -3e38)
                nc.vector.max(mxt[:, 8:16], Gm[:])
                nc.vector.max_index(mip[:, 8:16], mxt[:, 8:16], Gm[:])
                # it[p, t] = mip[t, p % 16], built via PE transpose + replication
                mipf = R.tile([128, 128], f32, name="mipf", tag="mipf")
                nc.scalar.copy(mipf[:], mip[:])
                tp1 = PS.tile([128, 128], f32, name="tp1", tag="psA", bufs=2,
                              padded_shape=[128, 512])
                nc.tensor.transpose(tp1[:], mipf[:], ident[:])
                mipT = R.tile([128, 128], f32, name="mipT", tag="mipT")
                nc.scalar.copy(mipT[:], tp1[:])
                tp2 = PS.tile([128, 128], f32, name="tp2", tag="psA", bufs=2,
                              padded_shape=[128, 512])
                nc.tensor.matmul(tp2[:], rep16[:], mipT[:], start=True, stop=True)
                it = idxts[rt % 2]
                nc.scalar.copy(it[:], tp2[:])
                return it

            def gather(rt, it):
                halves = []
                for s in range(2):
                    kvug = GP.tile([128, 1024 * 6], bf16, name="kvug", tag="kvug",
                                   bufs=3)
                    nc.gpsimd.ap_gather(
                        kvug[:].rearrange("p (n d) -> p n d", d=6),
                        kvu6v, it[:, s * 64 : (s + 1) * 64],
                        channels=128, num_elems=N, d=6, num_idxs=1024,
                    )
                    halves.append(kvug)
                return halves

            def pair_compute(rt, kvugs):
                t0 = rt * CH
                gvs = [kv[:].rearrange("p (n d) -> p n d", d=6) for kv in kvugs]
                eh = [FP.tile([128, 2048], bf16, name=f"eh{h}", tag=f"eh{h}", bufs=2)
                      for h in range(2)]
                wv = [FP.tile([128, 2048], bf16, name=f"wv{h}", tag=f"wv{h}", bufs=2)
                      for h in range(2)]
                for fc in range(NFC):
                    pr = slice(fc * FC, (fc + 1) * FC)
                    lpr = slice((fc % 2) * 512, (fc % 2) * 512 + 512)
                    g = gvs[fc // 2]
                    tsl = slice(t0 + fc * TPF, t0 + (fc + 1) * TPF)
                    # pe1 = relu(sw @ pos_n(bcast) + sw @ (-pos_g) + b1)
                    pe1t = [FP.tile([128, 512], bf16, name=f"pe1{h}_{fc}",
                                    tag=f"pe1{h}", bufs=2) for h in range(2)]
                    for h in range(2):
                        pp = PS.tile([128, 512], f32, name=f"pp{h}", tag=f"ab{h}", bufs=2)
                        nc.tensor.matmul(
                            pp[:], sw[:, h * 128 : (h + 1) * 128],
                            ppAb[0:4, tsl].unsqueeze(2).broadcast_to([4, TPF, K]),
                            start=True, stop=False,
                        )
                        nc.tensor.matmul(
                            pp[:], sw[:, h * 128 : (h + 1) * 128],
                            g[0:4, lpr, 4],
                            start=False, stop=True,
                        )
                        nc.scalar.activation(
                            pe1t[h][:], pp[:], AF.Relu,
                            bias=bs[:, h : h + 1], scale=1.0,
                        )
                    # pv = pm_w2 @ pe1 + I @ v_g   (stays in PSUM for wv mul)
                    pv_ps = []
                    for h in range(2):
                        pv = PS.tile([128, 512], f32, name=f"pv{h}", tag=f"pv{h}", bufs=1)
                        nc.tensor.matmul(
                            pv[:], wpair[0][:, h * 128 : (h + 1) * 128], pe1t[0][:],
                            start=True, stop=False,
                        )
                        nc.tensor.matmul(
                            pv[:], wpair[1][:, h * 128 : (h + 1) * 128], pe1t[1][:],
                            start=False, stop=False,
                        )
                        nc.tensor.matmul(
                            pv[:], identb[:], g[:, lpr, 2 + h],
                            start=False, stop=True,
                        )
                        pv_ps.append(pv)
                    # ac = Wc @ pe1 + I @ q~(bcast) + I @ (-k~_g)
                    a1r = []
                    for h in range(2):
                        ac = PS.tile([128, 512], f32, name=f"ac{h}", tag=f"ab{h}", bufs=2)
                        nc.tensor.matmul(
                            ac[:], wpair[0][:, 256 + h * 128 : 256 + (h + 1) * 128],
                            pe1t[0][:], start=True, stop=False,
                        )
                        nc.tensor.matmul(
                            ac[:], wpair[1][:, 256 + h * 128 : 256 + (h + 1) * 128],
                            pe1t[1][:], start=False, stop=False,
                        )
                        nc.tensor.matmul(
                            ac[:], identb[:],
                            qb[h][:, tsl].unsqueeze(2).broadcast_to([128, TPF, K]),
                            start=False, stop=False,
                        )
                        nc.tensor.matmul(
                            ac[:], identb[:], g[:, lpr, h],
                            start=False, stop=True,
                        )
                        ar = FP.tile([128, 512], bf16, name=f"ar{h}", tag=f"ar{h}", bufs=2)
                        nc.scalar.activation(ar[:], ac[:], AF.Relu, bias=0.0, scale=1.0)
                        a1r.append(ar)
                    # a2 = Wl @ ar ; eh = exp(a2) ; wv = eh * pv
                    for h in range(2):
                        a2 = PS.tile([128, 512], f32, name=f"a2_{h}", tag=f"ab{h}", bufs=2)
                        nc.tensor.matmul(
                            a2[:], wpair[0][:, 512 + h * 128 : 512 + (h + 1) * 128],
                            a1r[0][:], start=True, stop=False,
                        )
                        nc.tensor.matmul(
                            a2[:], wpair[1][:, 512 + h * 128 : 512 + (h + 1) * 128],
                            a1r[1][:], start=False, stop=True,
                        )
                        nc.scalar.activation(eh[h][:, pr], a2[:], AF.Exp, bias=0.0, scale=1.0)
                        nc.vector.tensor_mul(wv[h][:, pr], eh[h][:, pr], pv_ps[h][:])
                # reductions over K + normalize + output
                # Z (sum of eh) on DVE; agg (sum of wv) on PE via identity-
                # accumulate over the 16 strided k-slices.
                aggn = []
                for h in range(2):
                    Z = FP.tile([128, CH], f32, name=f"Z{h}", tag=f"Z{h}", bufs=2)
                    nc.vector.tensor_reduce(
                        Z[:], eh[h][:].rearrange("p (a b) -> p a b", b=K), AX.X, OP.add,
                    )
                    wvv = wv[h][:].rearrange("p (a b) -> p a b", b=K)
                    agg_ps = PS.tile([128, CH], f32, name=f"aggp{h}", tag=f"pv{h}",
                                     bufs=1, padded_shape=[128, 512])
                    for k in range(K):
                        nc.tensor.matmul(
                            agg_ps[:], identb[:], wvv[:, :, k],
                            start=(k == 0), stop=(k == K - 1),
                        )
                    rz = FP.tile([128, CH], f32, name=f"rz{h}", tag=f"rz{h}", bufs=2)
                    if RECIP_FAST:
                        nc.vector.reciprocal_approx_fast(rz[:], Z[:])
                    else:
                        nc.vector.reciprocal(rz[:], Z[:])
                    an = FP.tile([128, CH], bf16, name=f"an{h}", tag=f"an{h}", bufs=2)
                    nc.vector.tensor_mul(an[:], agg_ps[:], rz[:])
                    aggn.append(an)
                for h in range(2):
                    op_ = PS.tile([128, CH], f32, name=f"op{h}", tag=f"ab{h}", bufs=2,
                                  padded_shape=[128, 512])
                    nc.tensor.matmul(
                        op_[:], wf[0][:, h * 128 : (h + 1) * 128], aggn[0][:],
                        start=True, stop=False,
                    )
                    nc.tensor.matmul(
                        op_[:], wf[1][:, h * 128 : (h + 1) * 128], aggn[1][:],
                        start=False, stop=True,
                    )
                    ob = FP.tile([128, CH], f32, name=f"ob{h}", tag=f"ob{h}", bufs=2)
                    nc.scalar.activation(
                        ob[:], op_[:], AF.Identity, bias=bs[:, 2 + h : 3 + h], scale=1.0
                    )
                    nc.vector.tensor_add(ob[:], ob[:], xs[h][:, t0 : t0 + CH].bitcast(f32))
                    nc.sync.dma_start(outT_d[h * 128 : (h + 1) * 128, t0 : t0 + CH], ob[:])

            # ---------------- software-pipelined main loop ----------------
            it0 = scores_topk(0)
            kv_prev = gather(0, it0)
            for rt in range(1, RT):
                it = scores_topk(rt)
                kv = gather(rt, it)
                pair_compute(rt - 1, kv_prev)
                kv_prev = kv
            pair_compute(RT - 1, kv_prev)

    nc.compile()
    return nc


def _get_nc():
    global _NC
    if _NC is None:
        _NC = _build()
    return _NC


def _make_in_maps(inputs):
    f = lambda k: np.ascontiguousarray(np.asarray(inputs[k], dtype=np.float32))
    x, pos = f("x"), f("pos")
    Wq, Wk, Wv, Wf, bf = f("Wq"), f("Wk"), f("Wv"), f("Wf"), f("bf")
    pm_w1, pm_g1, pm_b1 = f("pm_w1"), f("pm_g1"), f("pm_b1")
    pm_m1, pm_v1, pm_w2 = f("pm_m1"), f("pm_v1"), f("pm_w2")
    am_w1, am_g1, am_b1 = f("am_w1"), f("am_g1"), f("am_b1")
    am_m1, am_v1, am_w2 = f("am_m1"), f("am_v1"), f("am_w2")

    scale1 = pm_g1 / np.sqrt(pm_v1 + EPS)
    spw1 = pm_w1 * scale1[:, None]                 # (D, 3)
    bias1 = pm_b1 - pm_m1 * scale1
    scaleA = am_g1 / np.sqrt(am_v1 + EPS)
    samw1 = am_w1 * scaleA[:, None]                # (D, D)
    biasA = am_b1 - am_m1 * scaleA

    Wqp = samw1 @ Wq                               # q~ projection
    Wkp = samw1 @ Wk                               # k~ projection (negated in table)
    Wc = samw1 @ pm_w2                             # pe -> a1 path
    Wl = am_w2 / 16.0

    wproj = np.ascontiguousarray(
        np.concatenate([Wqp.T, -Wkp.T, Wv.T], axis=1), dtype=np.float32
    )
    wpair = np.ascontiguousarray(
        np.concatenate([pm_w2.T, Wc.T, Wl.T], axis=1), dtype=np.float32
    )
    wfin = np.ascontiguousarray(Wf.T, dtype=np.float32)
    spw1T = np.zeros((4, D), np.float32)
    spw1T[0:3, :] = spw1.T
    rep16 = np.zeros((128, 128), np.float32)
    for p in range(128):
        rep16[p % 16, p] = 1.0
    rep4n = np.zeros((4, 128), np.float32)
    for p in range(128):
        rep4n[p % 4, p] = -1.0
    ident = np.eye(128, dtype=np.float32)
    biases = np.zeros((128, 6), np.float32)
    biases[:, 0], biases[:, 1] = bias1[:128], bias1[128:]
    biases[:, 2], biases[:, 3] = bf[:128], bf[128:]
    biases[:, 4], biases[:, 5] = biasA[:128], biasA[128:]

    in_maps = []
    for b in range(B):
        xT = np.ascontiguousarray(x[b].T)
        p = pos[b]
        sq = (p * p).sum(-1)
        pospack = np.zeros((8, N), np.float32)
        pospack[0:3, :] = p.T
        pospack[3, :] = -sq
        pospack[4:7, :] = 2.0 * p.T
        pospack[7, :] = 1.0
        in_maps.append(
            {
                "xT": xT,
                "pospack": pospack,
                "wproj": wproj,
                "wpair": wpair,
                "wfin": wfin,
                "spw1T": spw1T,
                "biases": biases,
                "rep16": rep16,
                "rep4n": rep4n,
                "ident": ident,
            }
        )
    return in_maps


def kernel(**inputs):
    nc = _get_nc()
    in_maps = _make_in_maps(inputs)
    res = bass_utils.run_bass_kernel_spmd(nc, in_maps, list(range(B)), trace=False)
    out = np.stack([np.asarray(res.results[b]["outT"]).T for b in range(B)])
    return np.ascontiguousarray(out.astype(np.float32))
